# revision 12
# baseline (speedup 1.0000x reference)
"""Trainium2 Bass kernel for nn_Attention_cfged_88527865905334.

Multi-head attention (8 heads, pruned q/k=512, v=640 channels) over
x[64,197,768], returning (out[64,197,768], attn[64,8,197,197]).

Strategy: data-parallel over batch across 8 NeuronCores (8 batches/core,
no collectives). Per core, a single fused Bass/Tile kernel:
  - x^T loaded with a transposed DRAM access pattern (512B columns)
  - qkv^T = W_qkv^T @ x^T (f32r matmuls, N=512 chunks)
  - per (batch, head): S = Q K^T (head-pair row-packed via base_partition),
    softmax row-wise (ACT exp + accumulated row-sum), attn batch-stored
    (one DMA per (batch, n-tile) covering all heads), attn^T re-read from
    DRAM with a transposed access pattern for the attn @ V matmul
  - out_u^T accumulated per head, DMA-compacted to 128-row tiles
  - proj = out_u @ W_proj + b_proj
All matmuls run in float32r (TF32-like, ~13-bit mantissa, full PE rate).
"""
import sys

if "/opt/trn_rl_repo" not in sys.path:
    sys.path.insert(0, "/opt/trn_rl_repo")

import numpy as np

# problem shapes (hardcoded per spec)
B = 64          # full batch
NB = 8          # batches per core
SEQ = 197
TOK = NB * SEQ          # 1576 tokens per core
TOKP = TOK + 2          # padded to even 1578
C = 768
H = 8
DQ = 64
DV = 80
CQK = 1024
CV = 640
CQKV = CQK + CV         # 1664
SCALE = DQ ** -0.5
N_CORES = 8

ATTN_BF16 = True  # bf16 attn-transport + V for the attn@V matmul (out err ~3e-3 vs 3e-4)

_CACHE = {}


def _build_nc():
    import concourse.bass as bass
    from concourse import bacc
    import concourse.tile as tile
    from concourse import mybir
    from concourse.tile import add_dep_helper
    from concourse.masks import make_identity

    f32 = mybir.dt.float32
    f32r = mybir.dt.float32r
    bf16 = mybir.dt.bfloat16
    tdt = bf16 if ATTN_BF16 else f32r
    EXP = mybir.ActivationFunctionType.Exp

    nc = bacc.Bacc("TRN2", target_bir_lowering=False, debug=False)
    x_d = nc.dram_tensor("x", [NB, SEQ, C], f32, kind="ExternalInput")
    wqkv_d = nc.dram_tensor("w_qkv", [C, CQKV], f32, kind="ExternalInput")
    wproj_d = nc.dram_tensor("w_proj", [CV, C], f32, kind="ExternalInput")
    b_d = nc.dram_tensor("b_proj", [C], f32, kind="ExternalInput")
    out_d = nc.dram_tensor("out", [NB, SEQ, C], f32, kind="ExternalOutput")
    attn_d = nc.dram_tensor("attn", [NB, H, SEQ, SEQ], f32, kind="ExternalOutput")

    x_flat = x_d.ap().flatten_outer_dims()      # [1576, 768]
    out_flat = out_d.ap().flatten_outer_dims()  # [1576, 768]
    attn_ap = attn_d.ap()

    KT = C // 128            # 6 k-tiles of x channels
    CT = CQKV // 128         # 13 c-tiles of qkv channels
    CVT = CV // 128          # 5 tiles of v channels
    NT = (TOK + 127) // 128  # 13 token tiles (last = 40)
    NCH = [(0, 512), (512, 512), (1024, 512), (1536, 42)]
    MSZ = [128, SEQ - 128]   # m/n subtile sizes: 128, 69

    def pcopy(i, out, in_):
        """psum->sbuf (or sbuf->sbuf) copy alternating DVE/ACT."""
        if i % 2 == 0:
            nc.vector.tensor_copy(out=out, in_=in_)
        else:
            nc.scalar.copy(out=out, in_=in_)

    with tile.TileContext(nc) as tc:
        with tc.tile_pool(name="const", bufs=1) as const:
            bias_bc = const.tile([128, C], f32)
            nc.sync.dma_start(
                out=bias_bc,
                in_=bass.AP(tensor=b_d.ap().tensor, offset=0, ap=[[0, 128], [1, C]]),
            )
            ident = const.tile([128, 128], f32)
            make_identity(nc, ident[:])
            ident_rt = const.tile([128, 128], f32r)
            nc.vector.tensor_copy(out=ident_rt[:], in_=ident[:])
            ident_r = ident_rt[:]
            ident_bt = const.tile([128, 128], bf16)
            nc.vector.tensor_copy(out=ident_bt[:], in_=ident[:])
            ident_b = ident_bt[:] if ATTN_BF16 else ident_r

            with tc.tile_pool(name="qkvT_p", bufs=1) as qkvT_p:
                qkvT = qkvT_p.tile([128, CT, TOKP], f32r)

                # ---- Phase 1+2: x^T (transposed DMA) and qkv^T = W^T x^T ----
                with (
                    tc.tile_pool(name="xt_p", bufs=1) as xt_p,
                    tc.tile_pool(name="x_st_p", bufs=2) as x_st_p,
                    tc.tile_pool(name="wq_st_p", bufs=2) as wq_st_p,
                    tc.tile_pool(name="wq_p", bufs=1) as wq_p,
                    tc.tile_pool(name="ps_qk", bufs=3, space="PSUM") as ps_qk,
                ):
                    xt = xt_p.tile([128, KT, TOKP], f32r)
                    nc.gpsimd.memset(xt[:, :, TOK:TOKP].bitcast(f32), 0.0)
                    for nt in range(NT):
                        nsz = min(128, TOK - nt * 128)
                        x_st = x_st_p.tile([128, C], f32, tag="x_st")
                        nc.sync.dma_start(
                            out=x_st[0:nsz, :],
                            in_=x_flat[nt * 128 : nt * 128 + nsz, :],
                        )
                        for kt in range(KT):
                            pt = ps_qk.tile([128, 128], f32, tag="ptr")
                            nc.tensor.transpose(
                                pt[:, 0:nsz],
                                x_st[0:nsz, kt * 128 : (kt + 1) * 128],
                                ident[0:nsz, 0:nsz],
                            )
                            pcopy(
                                nt * KT + kt,
                                xt[:, kt, nt * 128 : nt * 128 + nsz],
                                pt[:, 0:nsz],
                            )

                    wq = wq_p.tile([128, KT, CQKV], f32r)
                    for kt in range(KT):
                        wq_st = wq_st_p.tile([128, CQKV], f32, tag="wq_st")
                        nc.sync.dma_start(
                            out=wq_st,
                            in_=wqkv_d.ap()[kt * 128 : (kt + 1) * 128, :],
                        )
                        pcopy(kt + 1, wq[:, kt, :], wq_st[:])

                    for ct in range(CT):
                        for ich, (n0, nw) in enumerate(NCH):
                            pq = ps_qk.tile([128, 512], f32, tag="pq")
                            for kt in range(KT):
                                nc.tensor.matmul(
                                    pq[:, 0:nw],
                                    wq[:, kt, ct * 128 : (ct + 1) * 128],
                                    xt[:, kt, n0 : n0 + nw],
                                    start=(kt == 0),
                                    stop=(kt == KT - 1),
                                )
                            pcopy(ct * 4 + ich, qkvT[:, ct, n0 : n0 + nw], pq[:, 0:nw])

                # ---- Phase 3: attention per (batch, head) ----
                with (
                    tc.tile_pool(name="wp_p", bufs=1) as wp_p,
                    tc.tile_pool(name="outuT_p", bufs=1) as outuT_p,
                ):
                    with tc.tile_pool(name="wp_st_p", bufs=1) as wp_st_p:
                        wp_st = wp_st_p.tile([128, CVT, C], f32)
                        nc.sync.dma_start(
                            out=wp_st,
                            in_=wproj_d.ap().rearrange("(a b) m -> b a m", b=128),
                        )
                        wp = wp_p.tile([128, CVT, C], f32r)
                        for cvt in range(CVT):
                            pcopy(cvt, wp[:, cvt, :], wp_st[:, cvt, :])

                    outuT = outuT_p.tile([128, CVT, TOKP], f32r)

                    with (
                        tc.tile_pool(name="vb_p", bufs=2) as vb_p,
                        tc.tile_pool(name="at_p", bufs=6) as at_p,
                        tc.tile_pool(name="asb_p", bufs=2) as asb_p,
                        tc.tile_pool(name="atT_p", bufs=2) as atT_p,
                        tc.tile_pool(name="ps_s3", bufs=3, space="PSUM") as ps_s3,
                        tc.tile_pool(name="ps_tr3", bufs=3, space="PSUM") as ps_tr3,
                        tc.tile_pool(name="ps_av", bufs=2, space="PSUM") as ps_av,
                    ):
                        for b in range(NB):
                            c0 = b * SEQ
                            # V_b[m, c] via PE transpose of qkv^T v-rows
                            vb = vb_p.tile([128, 2, CV], tdt, tag="vb")
                            for m2 in range(2):
                                mm = MSZ[m2]
                                mc0 = c0 + m2 * 128
                                for cv in range(CVT):
                                    pt = ps_tr3.tile([128, 128], f32r, tag="ptr3", name="ptv")
                                    nc.tensor.transpose(
                                        pt[0:mm, :],
                                        qkvT[:, 8 + cv, mc0 : mc0 + mm],
                                        ident_r[:, :],
                                    )
                                    pcopy(
                                        m2 * CVT + cv,
                                        vb[0:mm, m2, cv * 128 : (cv + 1) * 128],
                                        pt[0:mm, :],
                                    )

                            # S + softmax, 8 heads into per-n-tile batch tiles
                            asb = [
                                asb_p.tile([128, H * SEQ], f32r, tag=f"asb{n2}", name=f"asb{n2}")
                                for n2 in range(2)
                            ]
                            for hp in range(4):
                                ps_pair = []
                                for n2 in range(2):
                                    for dh in range(2):
                                        h = 2 * hp + dh
                                        r0 = 64 * dh
                                        q_ap = qkvT[
                                            r0 : r0 + 64, h // 2, c0 : c0 + 198
                                        ]
                                        k_ap = qkvT[
                                            r0 : r0 + 64, 4 + h // 2, c0 : c0 + 198
                                        ]
                                        nn = MSZ[n2]
                                        p_s = ps_s3.tile([128, 198], f32, tag="ps3", name="p_s")
                                        nc.tensor.matmul(
                                            p_s[0:nn, :],
                                            q_ap[:, n2 * 128 : n2 * 128 + nn],
                                            k_ap[:],
                                            start=True,
                                            stop=True,
                                        )
                                        ps_pair.append((h, n2, nn, p_s))
                                for h, n2, nn, p_s in ps_pair:
                                    expS = at_p.tile([128, SEQ], f32, tag="expS")
                                    rs = at_p.tile([128, 2], f32, tag="rs")
                                    nc.scalar.activation(
                                        out=expS[0:nn, :],
                                        in_=p_s[0:nn, 0:SEQ],
                                        func=EXP,
                                        scale=SCALE,
                                        accum_out=rs[0:nn, 0:1],
                                    )
                                    nc.vector.reciprocal(
                                        out=rs[0:nn, 1:2], in_=rs[0:nn, 0:1]
                                    )
                                    nc.vector.tensor_scalar_mul(
                                        asb[n2][0:nn, h * SEQ : (h + 1) * SEQ],
                                        expS[0:nn, :],
                                        rs[0:nn, 1:2],
                                    )

                            if ATTN_BF16:
                                asb_t = [
                                    asb_p.tile(
                                        [128, H * SEQ], bf16,
                                        tag=f"asbt{n2}", name=f"asbt{n2}",
                                    )
                                    for n2 in range(2)
                                ]
                                for n2 in range(2):
                                    nc.vector.tensor_copy(
                                        out=asb_t[n2][0 : MSZ[n2], :],
                                        in_=asb[n2][0 : MSZ[n2], :],
                                    )
                            else:
                                asb_t = asb

                            # store attn: one DMA per n-tile covering all heads
                            st_insts = []
                            attn_b = attn_ap[b].rearrange("h n m -> n h m")
                            for n2 in range(2):
                                nn = MSZ[n2]
                                st = nc.sync.dma_start(
                                    out=attn_b[n2 * 128 : n2 * 128 + nn],
                                    in_=asb[n2][0:nn, :]
                                    .bitcast(f32)
                                    .rearrange("p (h m) -> p h m", h=H),
                                )
                                st_insts.append(st)

                            # attn^T via PE transposes of the normalized attn
                            atT = []
                            for m2 in range(2):
                                mm = MSZ[m2]
                                t3 = atT_p.tile(
                                    [128, H, 198], tdt, tag=f"atT{m2}", name=f"atT{m2}"
                                )
                                atT.append(t3)
                            for h in range(H):
                                for m2 in range(2):
                                    mm = MSZ[m2]
                                    for n2 in range(2):
                                        nn = MSZ[n2]
                                        nn_p = nn + (nn % 2)  # pad odd N for f32r
                                        pt = ps_tr3.tile(
                                            [128, 128], tdt, tag="ptr3", name="pta"
                                        )
                                        nc.tensor.transpose(
                                            pt[0:mm, 0:nn_p],
                                            asb_t[n2][
                                                0:nn,
                                                h * SEQ + m2 * 128 : h * SEQ
                                                + m2 * 128 + mm,
                                            ],
                                            ident_b[0:nn, 0:nn_p]
                                            if ATTN_BF16
                                            else ident_r[0:nn, 0:nn_p],
                                        )
                                        pcopy(
                                            m2 + n2,
                                            atT[m2][0:mm, h, n2 * 128 : n2 * 128 + nn],
                                            pt[0:mm, 0:nn],
                                        )

                            for m2 in range(2):
                                if ATTN_BF16:
                                    nc.gpsimd.memset(atT[m2][:, :, 197:198], 0.0)
                                else:
                                    nc.gpsimd.memset(
                                        atT[m2][:, :, 197:198].bitcast(f32), 0.0
                                    )
                            # out_u^T[dv, n] += V^T attn^T, per head
                            for h in range(H):
                                p_av = ps_av.tile([128, 198], f32, tag="pav")
                                for m2 in range(2):
                                    mm = MSZ[m2]
                                    nc.tensor.matmul(
                                        p_av[0:DV, :],
                                        vb[0:mm, m2, h * DV : (h + 1) * DV],
                                        atT[m2][0:mm, h, 0:198],
                                        start=(m2 == 0),
                                        stop=(m2 == 1),
                                    )
                                avst = at_p.tile([128, SEQ], f32r, tag="avst")
                                pcopy(h, avst[0:DV, :], p_av[0:DV, 0:SEQ])
                                g0 = h * DV
                                t0, o0 = g0 // 128, g0 % 128
                                sz1 = min(128 - o0, DV)
                                nc.gpsimd.dma_start(
                                    out=outuT[o0 : o0 + sz1, t0, c0 : c0 + SEQ],
                                    in_=avst[0:sz1, 0:SEQ],
                                )
                                if sz1 < DV:
                                    nc.gpsimd.dma_start(
                                        out=outuT[0 : DV - sz1, t0 + 1, c0 : c0 + SEQ],
                                        in_=avst[sz1:DV, 0:SEQ],
                                    )

                    # ---- Phase 4: proj ----
                    with (
                        tc.tile_pool(name="out_p", bufs=3) as out_p,
                        tc.tile_pool(name="ps_pp", bufs=3, space="PSUM") as ps_pp,
                    ):
                        for nt in range(NT):
                            nsz = min(128, TOK - nt * 128)
                            o_st = out_p.tile([128, C], f32, tag="o_st")
                            for ch in range(2):
                                pp = ps_pp.tile([128, 384], f32, tag="pp")
                                for cvt in range(CVT):
                                    nc.tensor.matmul(
                                        pp[0:nsz, :],
                                        outuT[:, cvt, nt * 128 : nt * 128 + nsz],
                                        wp[:, cvt, ch * 384 : (ch + 1) * 384],
                                        start=(cvt == 0),
                                        stop=(cvt == CVT - 1),
                                    )
                                nc.vector.tensor_add(
                                    o_st[0:nsz, ch * 384 : (ch + 1) * 384],
                                    pp[0:nsz, :],
                                    bias_bc[0:nsz, ch * 384 : (ch + 1) * 384],
                                )
                            nc.sync.dma_start(
                                out=out_flat[nt * 128 : nt * 128 + nsz, :],
                                in_=o_st[0:nsz, :],
                            )

    nc.compile()
    return nc


def _get_nc():
    if "nc" not in _CACHE:
        _CACHE["nc"] = _build_nc()
    return _CACHE["nc"]


def kernel(x, W_qkv, W_proj, b_proj, _trace=False, _tmpdir=None):
    from concourse.bass_utils import run_bass_kernel_spmd

    x = np.asarray(x, dtype=np.float32)
    W_qkv = np.asarray(W_qkv, dtype=np.float32)
    W_proj = np.asarray(W_proj, dtype=np.float32)
    b_proj = np.asarray(b_proj, dtype=np.float32)

    nc = _get_nc()
    in_maps = [
        {
            "x": np.ascontiguousarray(x[i * NB : (i + 1) * NB]),
            "w_qkv": W_qkv,
            "w_proj": W_proj,
            "b_proj": b_proj,
        }
        for i in range(N_CORES)
    ]
    kw = {}
    if _trace:
        kw = dict(trace=True, tmpdir=_tmpdir)
    res = run_bass_kernel_spmd(nc, in_maps, core_ids=list(range(N_CORES)), **kw)
    out = np.concatenate([res.results[i]["out"] for i in range(N_CORES)], axis=0)
    attn = np.concatenate([res.results[i]["attn"] for i in range(N_CORES)], axis=0)
    if _trace:
        return (out, attn), res
    return (out, attn)


# revision 13
# speedup vs baseline: 1.1066x; 1.1066x over previous
"""Trainium2 Bass kernel for nn_Attention_cfged_88527865905334.

Multi-head attention (8 heads, pruned q/k=512, v=640 channels) over
x[64,197,768], returning (out[64,197,768], attn[64,8,197,197]).

Strategy: data-parallel over batch across 8 NeuronCores (8 batches/core,
no collectives). Per core, a single fused Bass/Tile kernel:
  - x^T loaded with a transposed DRAM access pattern (512B columns)
  - qkv^T = W_qkv^T @ x^T (f32r matmuls, N=512 chunks)
  - per (batch, head): S = Q K^T (head-pair row-packed via base_partition),
    softmax row-wise (ACT exp + accumulated row-sum), attn batch-stored
    (one DMA per (batch, n-tile) covering all heads), attn^T re-read from
    DRAM with a transposed access pattern for the attn @ V matmul
  - out_u^T accumulated per head, DMA-compacted to 128-row tiles
  - proj = out_u @ W_proj + b_proj
All matmuls run in float32r (TF32-like, ~13-bit mantissa, full PE rate).
"""
import sys

if "/opt/trn_rl_repo" not in sys.path:
    sys.path.insert(0, "/opt/trn_rl_repo")

import numpy as np

# problem shapes (hardcoded per spec)
B = 64          # full batch
NB = 8          # batches per core
SEQ = 197
TOK = NB * SEQ          # 1576 tokens per core
TOKP = TOK + 2          # padded to even 1578
C = 768
H = 8
DQ = 64
DV = 80
CQK = 1024
CV = 640
CQKV = CQK + CV         # 1664
SCALE = DQ ** -0.5
N_CORES = 8

ATTN_BF16 = False  # bf16 attn-transport + V for the attn@V matmul (out err ~3e-3 vs 3e-4)

_CACHE = {}


def _build_nc():
    import concourse.bass as bass
    from concourse import bacc
    import concourse.tile as tile
    from concourse import mybir
    from concourse.tile import add_dep_helper
    from concourse.masks import make_identity

    f32 = mybir.dt.float32
    f32r = mybir.dt.float32r
    bf16 = mybir.dt.bfloat16
    tdt = bf16 if ATTN_BF16 else f32r
    EXP = mybir.ActivationFunctionType.Exp

    nc = bacc.Bacc("TRN2", target_bir_lowering=False, debug=False)
    x_d = nc.dram_tensor("x", [NB, SEQ, C], f32, kind="ExternalInput")
    wqkv_d = nc.dram_tensor("w_qkv", [C, CQKV], f32, kind="ExternalInput")
    wproj_d = nc.dram_tensor("w_proj", [CV, C], f32, kind="ExternalInput")
    b_d = nc.dram_tensor("b_proj", [C], f32, kind="ExternalInput")
    out_d = nc.dram_tensor("out", [NB, SEQ, C], f32, kind="ExternalOutput")
    attn_d = nc.dram_tensor("attn", [NB, H, SEQ, SEQ], f32, kind="ExternalOutput")

    x_flat = x_d.ap().flatten_outer_dims()      # [1576, 768]
    out_flat = out_d.ap().flatten_outer_dims()  # [1576, 768]
    attn_ap = attn_d.ap()

    KT = C // 128            # 6 k-tiles of x channels
    CT = CQKV // 128         # 13 c-tiles of qkv channels
    CVT = CV // 128          # 5 tiles of v channels
    NT = (TOK + 127) // 128  # 13 token tiles (last = 40)
    NCH = [(0, 512), (512, 512), (1024, 512), (1536, 42)]
    MSZ = [128, SEQ - 128]   # m/n subtile sizes: 128, 69

    def pcopy(i, out, in_):
        """psum->sbuf (or sbuf->sbuf) copy alternating DVE/ACT."""
        if i % 2 == 0:
            nc.vector.tensor_copy(out=out, in_=in_)
        else:
            nc.scalar.copy(out=out, in_=in_)

    with tile.TileContext(nc) as tc:
        with tc.tile_pool(name="const", bufs=1) as const:
            bias_bc = const.tile([128, C], f32)
            nc.sync.dma_start(
                out=bias_bc,
                in_=bass.AP(tensor=b_d.ap().tensor, offset=0, ap=[[0, 128], [1, C]]),
            )
            ident = const.tile([128, 128], f32)
            make_identity(nc, ident[:])
            ident_rt = const.tile([128, 128], f32r)
            nc.vector.tensor_copy(out=ident_rt[:], in_=ident[:])
            ident_r = ident_rt[:]
            ident_bt = const.tile([128, 128], bf16)
            nc.vector.tensor_copy(out=ident_bt[:], in_=ident[:])
            ident_b = ident_bt[:] if ATTN_BF16 else ident_r

            with tc.tile_pool(name="qkvT_p", bufs=1) as qkvT_p:
                qkvT = qkvT_p.tile([128, CT, TOKP], f32r)

                # ---- Phase 1+2: x^T (transposed DMA) and qkv^T = W^T x^T ----
                with (
                    tc.tile_pool(name="xt_p", bufs=1) as xt_p,
                    tc.tile_pool(name="x_st_p", bufs=2) as x_st_p,
                    tc.tile_pool(name="wq_st_p", bufs=2) as wq_st_p,
                    tc.tile_pool(name="wq_p", bufs=1) as wq_p,
                    tc.tile_pool(name="ps_qk", bufs=3, space="PSUM") as ps_qk,
                ):
                    xt = xt_p.tile([128, KT, TOKP], f32r)
                    nc.gpsimd.memset(xt[:, :, TOK:TOKP].bitcast(f32), 0.0)
                    for nt in range(NT):
                        nsz = min(128, TOK - nt * 128)
                        x_st = x_st_p.tile([128, C], f32, tag="x_st")
                        nc.sync.dma_start(
                            out=x_st[0:nsz, :],
                            in_=x_flat[nt * 128 : nt * 128 + nsz, :],
                        )
                        for kt in range(KT):
                            pt = ps_qk.tile([128, 128], f32, tag="ptr")
                            nc.tensor.transpose(
                                pt[:, 0:nsz],
                                x_st[0:nsz, kt * 128 : (kt + 1) * 128],
                                ident[0:nsz, 0:nsz],
                            )
                            pcopy(
                                nt * KT + kt,
                                xt[:, kt, nt * 128 : nt * 128 + nsz],
                                pt[:, 0:nsz],
                            )

                    wq = wq_p.tile([128, KT, CQKV], f32r)
                    for kt in range(KT):
                        wq_st = wq_st_p.tile([128, CQKV], f32, tag="wq_st")
                        nc.sync.dma_start(
                            out=wq_st,
                            in_=wqkv_d.ap()[kt * 128 : (kt + 1) * 128, :],
                        )
                        pcopy(kt + 1, wq[:, kt, :], wq_st[:])

                    for ich, (n0, nw) in enumerate(NCH):
                        for ct in range(CT):
                            pq = ps_qk.tile([128, 512], f32, tag="pq")
                            for kt in range(KT):
                                nc.tensor.matmul(
                                    pq[:, 0:nw],
                                    wq[:, kt, ct * 128 : (ct + 1) * 128],
                                    xt[:, kt, n0 : n0 + nw],
                                    start=(kt == 0),
                                    stop=(kt == KT - 1),
                                )
                            pcopy(ct * 4 + ich, qkvT[:, ct, n0 : n0 + nw], pq[:, 0:nw])

                # ---- Phase 3: attention per (batch, head) ----
                with (
                    tc.tile_pool(name="wp_p", bufs=1) as wp_p,
                    tc.tile_pool(name="outuT_p", bufs=1) as outuT_p,
                ):
                    with tc.tile_pool(name="wp_st_p", bufs=1) as wp_st_p:
                        wp_st = wp_st_p.tile([128, CVT, C], f32)
                        nc.sync.dma_start(
                            out=wp_st,
                            in_=wproj_d.ap().rearrange("(a b) m -> b a m", b=128),
                        )
                        wp = wp_p.tile([128, CVT, C], f32r)
                        for cvt in range(CVT):
                            pcopy(cvt, wp[:, cvt, :], wp_st[:, cvt, :])

                    outuT = outuT_p.tile([128, CVT, TOKP], f32r)

                    with (
                        tc.tile_pool(name="vb_p", bufs=2) as vb_p,
                        tc.tile_pool(name="at_p", bufs=6) as at_p,
                        tc.tile_pool(name="asb_p", bufs=2) as asb_p,
                        tc.tile_pool(name="atT_p", bufs=2) as atT_p,
                        tc.tile_pool(name="ps_s3", bufs=3, space="PSUM") as ps_s3,
                        tc.tile_pool(name="ps_tr3", bufs=3, space="PSUM") as ps_tr3,
                        tc.tile_pool(name="ps_av", bufs=2, space="PSUM") as ps_av,
                    ):
                        for b in range(NB):
                            c0 = b * SEQ
                            # V_b[m, c] via PE transpose of qkv^T v-rows
                            vb = vb_p.tile([128, 2, CV], tdt, tag="vb")
                            for m2 in range(2):
                                mm = MSZ[m2]
                                mc0 = c0 + m2 * 128
                                for cv in range(CVT):
                                    pt = ps_tr3.tile([128, 128], f32r, tag="ptr3", name="ptv")
                                    nc.tensor.transpose(
                                        pt[0:mm, :],
                                        qkvT[:, 8 + cv, mc0 : mc0 + mm],
                                        ident_r[:, :],
                                    )
                                    pcopy(
                                        m2 * CVT + cv,
                                        vb[0:mm, m2, cv * 128 : (cv + 1) * 128],
                                        pt[0:mm, :],
                                    )

                            # S + softmax, 8 heads into per-n-tile batch tiles
                            asb = [
                                asb_p.tile([128, H * SEQ], f32r, tag=f"asb{n2}", name=f"asb{n2}")
                                for n2 in range(2)
                            ]
                            for hp in range(4):
                                ps_pair = []
                                for n2 in range(2):
                                    for dh in range(2):
                                        h = 2 * hp + dh
                                        r0 = 64 * dh
                                        q_ap = qkvT[
                                            r0 : r0 + 64, h // 2, c0 : c0 + 198
                                        ]
                                        k_ap = qkvT[
                                            r0 : r0 + 64, 4 + h // 2, c0 : c0 + 198
                                        ]
                                        nn = MSZ[n2]
                                        p_s = ps_s3.tile([128, 198], f32, tag="ps3", name="p_s")
                                        nc.tensor.matmul(
                                            p_s[0:nn, :],
                                            q_ap[:, n2 * 128 : n2 * 128 + nn],
                                            k_ap[:],
                                            start=True,
                                            stop=True,
                                        )
                                        ps_pair.append((h, n2, nn, p_s))
                                for h, n2, nn, p_s in ps_pair:
                                    expS = at_p.tile([128, SEQ], f32, tag="expS")
                                    rs = at_p.tile([128, 2], f32, tag="rs")
                                    nc.scalar.activation(
                                        out=expS[0:nn, :],
                                        in_=p_s[0:nn, 0:SEQ],
                                        func=EXP,
                                        scale=SCALE,
                                        accum_out=rs[0:nn, 0:1],
                                    )
                                    nc.vector.reciprocal(
                                        out=rs[0:nn, 1:2], in_=rs[0:nn, 0:1]
                                    )
                                    nc.vector.tensor_scalar_mul(
                                        asb[n2][0:nn, h * SEQ : (h + 1) * SEQ],
                                        expS[0:nn, :],
                                        rs[0:nn, 1:2],
                                    )

                            if ATTN_BF16:
                                asb_t = [
                                    asb_p.tile(
                                        [128, H * SEQ], bf16,
                                        tag=f"asbt{n2}", name=f"asbt{n2}",
                                    )
                                    for n2 in range(2)
                                ]
                                for n2 in range(2):
                                    nc.vector.tensor_copy(
                                        out=asb_t[n2][0 : MSZ[n2], :],
                                        in_=asb[n2][0 : MSZ[n2], :],
                                    )
                            else:
                                asb_t = asb

                            # store attn: one DMA per n-tile covering all heads
                            st_insts = []
                            attn_b = attn_ap[b].rearrange("h n m -> n h m")
                            for n2 in range(2):
                                nn = MSZ[n2]
                                st = nc.sync.dma_start(
                                    out=attn_b[n2 * 128 : n2 * 128 + nn],
                                    in_=asb[n2][0:nn, :]
                                    .bitcast(f32)
                                    .rearrange("p (h m) -> p h m", h=H),
                                )
                                st_insts.append(st)

                            # attn^T via PE transposes of the normalized attn
                            atT = []
                            for m2 in range(2):
                                mm = MSZ[m2]
                                t3 = atT_p.tile(
                                    [128, H, 198], tdt, tag=f"atT{m2}", name=f"atT{m2}"
                                )
                                atT.append(t3)
                            for h in range(H):
                                for m2 in range(2):
                                    mm = MSZ[m2]
                                    for n2 in range(2):
                                        nn = MSZ[n2]
                                        nn_p = nn + (nn % 2)  # pad odd N for f32r
                                        pt = ps_tr3.tile(
                                            [128, 128], tdt, tag="ptr3", name="pta"
                                        )
                                        nc.tensor.transpose(
                                            pt[0:mm, 0:nn_p],
                                            asb_t[n2][
                                                0:nn,
                                                h * SEQ + m2 * 128 : h * SEQ
                                                + m2 * 128 + mm,
                                            ],
                                            ident_b[0:nn, 0:nn_p]
                                            if ATTN_BF16
                                            else ident_r[0:nn, 0:nn_p],
                                        )
                                        pcopy(
                                            m2 + n2,
                                            atT[m2][0:mm, h, n2 * 128 : n2 * 128 + nn],
                                            pt[0:mm, 0:nn],
                                        )

                            for m2 in range(2):
                                if ATTN_BF16:
                                    nc.gpsimd.memset(atT[m2][:, :, 197:198], 0.0)
                                else:
                                    nc.gpsimd.memset(
                                        atT[m2][:, :, 197:198].bitcast(f32), 0.0
                                    )
                            # out_u^T[dv, n] += V^T attn^T, per head
                            for h in range(H):
                                p_av = ps_av.tile([128, 198], f32, tag="pav")
                                for m2 in range(2):
                                    mm = MSZ[m2]
                                    nc.tensor.matmul(
                                        p_av[0:DV, :],
                                        vb[0:mm, m2, h * DV : (h + 1) * DV],
                                        atT[m2][0:mm, h, 0:198],
                                        start=(m2 == 0),
                                        stop=(m2 == 1),
                                    )
                                avst = at_p.tile([128, SEQ], f32r, tag="avst")
                                pcopy(h, avst[0:DV, :], p_av[0:DV, 0:SEQ])
                                g0 = h * DV
                                t0, o0 = g0 // 128, g0 % 128
                                sz1 = min(128 - o0, DV)
                                nc.gpsimd.dma_start(
                                    out=outuT[o0 : o0 + sz1, t0, c0 : c0 + SEQ],
                                    in_=avst[0:sz1, 0:SEQ],
                                )
                                if sz1 < DV:
                                    nc.gpsimd.dma_start(
                                        out=outuT[0 : DV - sz1, t0 + 1, c0 : c0 + SEQ],
                                        in_=avst[sz1:DV, 0:SEQ],
                                    )

                    # ---- Phase 4: proj ----
                    with (
                        tc.tile_pool(name="out_p", bufs=3) as out_p,
                        tc.tile_pool(name="ps_pp", bufs=3, space="PSUM") as ps_pp,
                    ):
                        for nt in range(NT):
                            nsz = min(128, TOK - nt * 128)
                            o_st = out_p.tile([128, C], f32, tag="o_st")
                            for ch in range(2):
                                pp = ps_pp.tile([128, 384], f32, tag="pp")
                                for cvt in range(CVT):
                                    nc.tensor.matmul(
                                        pp[0:nsz, :],
                                        outuT[:, cvt, nt * 128 : nt * 128 + nsz],
                                        wp[:, cvt, ch * 384 : (ch + 1) * 384],
                                        start=(cvt == 0),
                                        stop=(cvt == CVT - 1),
                                    )
                                nc.vector.tensor_add(
                                    o_st[0:nsz, ch * 384 : (ch + 1) * 384],
                                    pp[0:nsz, :],
                                    bias_bc[0:nsz, ch * 384 : (ch + 1) * 384],
                                )
                            nc.sync.dma_start(
                                out=out_flat[nt * 128 : nt * 128 + nsz, :],
                                in_=o_st[0:nsz, :],
                            )

    nc.compile()
    return nc


def _get_nc():
    if "nc" not in _CACHE:
        _CACHE["nc"] = _build_nc()
    return _CACHE["nc"]


def kernel(x, W_qkv, W_proj, b_proj, _trace=False, _tmpdir=None):
    from concourse.bass_utils import run_bass_kernel_spmd

    x = np.asarray(x, dtype=np.float32)
    W_qkv = np.asarray(W_qkv, dtype=np.float32)
    W_proj = np.asarray(W_proj, dtype=np.float32)
    b_proj = np.asarray(b_proj, dtype=np.float32)

    nc = _get_nc()
    in_maps = [
        {
            "x": np.ascontiguousarray(x[i * NB : (i + 1) * NB]),
            "w_qkv": W_qkv,
            "w_proj": W_proj,
            "b_proj": b_proj,
        }
        for i in range(N_CORES)
    ]
    kw = {}
    if _trace:
        kw = dict(trace=True, tmpdir=_tmpdir)
    res = run_bass_kernel_spmd(nc, in_maps, core_ids=list(range(N_CORES)), **kw)
    out = np.concatenate([res.results[i]["out"] for i in range(N_CORES)], axis=0)
    attn = np.concatenate([res.results[i]["attn"] for i in range(N_CORES)], axis=0)
    if _trace:
        return (out, attn), res
    return (out, attn)
